# revision 8
# baseline (speedup 1.0000x reference)
"""ConvAttention Trainium2 kernel (nn_ConvAttention_61083024883911).

Strategy: data-parallel over batch (8 samples/core on 8 cores), each core
processes its 8 samples as 4 token-pairs (2 samples side by side -> 394-wide
free dim so float32r matmuls run at full PE rate).

Everything on-device lives channel-major [d, tokens] so softmax, the
depthwise convs, and the elementwise ops all stream along the free axis:
  - host pre-transposes x -> xT and pw1/pw2 -> pwT (no on-device transposes)
  - qkv^T = w_qkv.T @ xT via PE (f32r), heads land on partitions
  - softmax-over-tokens: ACT exp with fused accum (row sums), no max-subtract
    (k values are O(1), exp cannot overflow in fp32)
  - ctx_diag via fused DVE tensor_tensor_reduce (expk * vT, row-reduce)
  - depthwise 3x3 via 9 DVE multiply-accumulate taps on zero-padded
    [128, 2, 16, 16] windows (per-partition scalar weights)
  - pointwise convs are plain matmuls; conv1 path in bf16 (its output is
    tiny vs the residual), conv2 path f32r
  - final out = out_pre^T.T @ w_out with the bias folded in as a k=1 matmul
"""

import numpy as np

B, N, D = 64, 197, 768
HEADS, DH = 12, 64
INNER = HEADS * DH
HW = 14
SP = HW * HW  # 196 spatial positions
NCORES = 8
BL = B // NCORES  # samples per core
NPAIR = BL // 2   # sample pairs per core
PW = 2 * N        # free width of one pair (394)
KC = D // 128     # 6 contraction chunks
MC_QKV = 3 * INNER // 128  # 18 output chunks for qkv

_cache = {}


def _patch_tile_drain():
    """This walrus build rejects Drain instructions with >1 sem wait ("Too
    many sync wait commands").  Split the TileContext tail-drain waits onto
    individual SP NOPs (one wait each) instead."""
    import concourse.mybir as mybir
    import concourse.tile as tile
    from concourse.vector_clock import ScopedClock

    if getattr(tile.TileContext, "_drain_patched", False):
        return

    def _drain_and_barrier(self, tick_clock, wait_clock):
        nc = self.nc
        probe = nc.sync.nop(nofuse=True)
        wait_clock.add_sem_waits(
            probe.ins, ScopedClock({None: tick_clock.global_clock})
        )
        si = probe.ins.sync_info
        waits = list(si.on_wait) if si is not None else []
        updates = list(si.on_update) if si is not None else []
        probe.ins.sync_info = mybir.SyncInfo(on_wait=waits[:1], on_update=updates)
        for w in waits[1:]:
            n = nc.sync.nop(nofuse=True)
            n.ins.sync_info = mybir.SyncInfo(on_wait=[w], on_update=[])
        nc.sync.drain()
        nc.all_engine_barrier()
        assert self.sems is not None
        popped = nc._tile_sem_poison_stack.pop()
        assert popped is self._sem_poison
        nc.clear_and_free_semaphores(list(self.sems.allocated().values()))
        nc.all_engine_barrier()

    tile.TileContext._drain_and_barrier = _drain_and_barrier
    tile.TileContext._drain_patched = True

    _patch_wait_split()


def _patch_wait_split():
    """This walrus build allows only ONE sem wait per instruction (any
    engine struct).  Post-process the serialized BIR: for every instruction
    carrying N>1 waits, insert N-1 same-engine NOPs immediately before it,
    each carrying one of the extra waits."""
    import orjson

    import concourse.bass as bass

    if getattr(bass.Bass, "_wait_split_patched", False):
        return
    orig = bass.Bass.to_json_bytes

    def to_json_bytes(self):
        j = orjson.loads(orig(self))
        n_split = 0
        for fn in j.get("functions", []):
            for blk in fn.get("blocks", []):
                ins_list = blk.get("instructions")
                if not ins_list:
                    continue
                out = []
                for ins in ins_list:
                    si = ins.get("sync_info")
                    waits = (si or {}).get("on_wait") or []
                    if len(waits) > 1:
                        for k, w in enumerate(waits[:-1]):
                            out.append({
                                "engine": ins["engine"],
                                "ins": [],
                                "name": f"{ins['name']}__wsplit{k}",
                                "opcode": "NoOp",
                                "outs": [],
                                "sync_info": {"on_update": [], "on_wait": [w]},
                            })
                            n_split += 1
                        si["on_wait"] = [waits[-1]]
                    out.append(ins)
                blk["instructions"] = out
        return orjson.dumps(j)

    bass.Bass.to_json_bytes = to_json_bytes
    bass.Bass._wait_split_patched = True


def _build(f32r=True, conv1_bf16=True, conv2_bf16=False):
    import concourse.bass as bass
    import concourse.mybir as mybir
    import concourse.tile as tile

    _patch_tile_drain()

    f32 = mybir.dt.float32
    bf16 = mybir.dt.bfloat16
    mmdt = mybir.dt.float32r if f32r else f32
    Alu = mybir.AluOpType
    Act = mybir.ActivationFunctionType
    c1dt = bf16 if conv1_bf16 else f32
    c2dt = bf16 if conv2_bf16 else f32

    # Tiles feeding f32r matmuls are declared float32r so every producer
    # (DMA cast via SWDGE, ACT/DVE writes) rounds-on-write, which the
    # walrus FP32r verifier requires.  mcast is then a no-op.
    def mcast(ap):
        return ap

    nc = bass.Bass("TRN2", target_bir_lowering=False, debug=False)

    # ---- DRAM I/O ----
    xT_d = nc.dram_tensor("xT", [NPAIR, KC, 128, PW], f32, kind="ExternalInput")
    wqkv_d = nc.dram_tensor("w_qkv", [KC, 128, 3 * INNER], f32, kind="ExternalInput")
    pw1T_d = nc.dram_tensor("pw1T", [KC, 128, D], f32, kind="ExternalInput")
    pw2T_d = nc.dram_tensor("pw2T", [KC, 128, D], f32, kind="ExternalInput")
    wout_d = nc.dram_tensor("w_out", [KC, 128, D], f32, kind="ExternalInput")
    dw1w_d = nc.dram_tensor("dw1w", [128, KC, 9], f32, kind="ExternalInput")
    dw2w_d = nc.dram_tensor("dw2w", [128, KC, 9], f32, kind="ExternalInput")
    bias_d = nc.dram_tensor("biases", [128, 4 * KC], f32, kind="ExternalInput")
    bout_d = nc.dram_tensor("b_out", [1, D], f32, kind="ExternalInput")
    ones_d = nc.dram_tensor("ones", [1, 128], f32, kind="ExternalInput")

    out_d = nc.dram_tensor("out", [BL * N, D], f32, kind="ExternalOutput")
    attn_d = nc.dram_tensor("attn", [NPAIR, KC, 128, PW], f32, kind="ExternalOutput")

    with tile.TileContext(nc) as tc:
        with (
            tc.tile_pool(name="weights", bufs=1) as wpool,
            tc.tile_pool(name="pads", bufs=1) as padpool,
            tc.tile_pool(name="acts", bufs=1) as apool,
            tc.tile_pool(name="expk", bufs=2) as epool,
            tc.tile_pool(name="outn", bufs=2) as opool,
            tc.tile_pool(name="xin", bufs=2) as xpool,
            tc.tile_pool(name="psmm", bufs=5, space="PSUM") as pmm,
            tc.tile_pool(name="psout", bufs=2, space="PSUM") as pout,
        ):
            # ---- persistent weights ----
            wqkv = wpool.tile([128, KC, 3 * INNER], mmdt)
            nc.gpsimd.dma_start(wqkv[:], xT_rearr(wqkv_d))
            pw1T = wpool.tile([128, KC, D], c1dt)
            if conv1_bf16:
                nc.gpsimd.dma_start(pw1T[:], xT_rearr(pw1T_d))  # SWDGE casts
            else:
                nc.sync.dma_start(pw1T[:], xT_rearr(pw1T_d))
            pw2T = wpool.tile([128, KC, D], bf16 if conv2_bf16 else mmdt)
            nc.gpsimd.dma_start(pw2T[:], xT_rearr(pw2T_d))
            wout = wpool.tile([128, KC, D], mmdt)
            nc.gpsimd.dma_start(wout[:], xT_rearr(wout_d))
            dw1w = wpool.tile([128, KC, 9], f32)
            nc.sync.dma_start(dw1w[:], dw1w_d.ap()[:])
            dw2w = wpool.tile([128, KC, 9], f32)
            nc.sync.dma_start(dw2w[:], dw2w_d.ap()[:])
            biases = wpool.tile([128, 4 * KC], f32)  # dw1_b | pw1_b | dw2_b | pw2_b
            nc.sync.dma_start(biases[:], bias_d.ap()[:])
            bout = wpool.tile([1, D], mmdt)
            nc.gpsimd.dma_start(bout[:], bout_d.ap()[:])
            ones = wpool.tile([1, 128], mmdt)
            nc.gpsimd.dma_start(ones[:], ones_d.ap()[:])

            dw1b = biases[:, 0:KC]
            pw1b = biases[:, KC:2 * KC]
            dw2b = biases[:, 2 * KC:3 * KC]
            pw2b = biases[:, 3 * KC:4 * KC]

            # ---- persistent padded conv windows (borders stay zero) ----
            pads1 = []
            pads2 = []
            for cc in range(KC):
                p1 = padpool.tile([128, 2, 16, 16], c1dt, name=f"pad1_{cc}",
                                  tag=f"pad1_{cc}")
                nc.gpsimd.memset(p1[:], 0.0)
                pads1.append(p1)
                p2 = padpool.tile([128, 2, 16, 16], c2dt, name=f"pad2_{cc}",
                                  tag=f"pad2_{cc}")
                nc.gpsimd.memset(p2[:], 0.0)
                pads2.append(p2)

            for p in range(NPAIR):
                # ---- load xT for the pair ----
                xa = xpool.tile([128, KC, PW], mmdt, tag="xa")
                nc.gpsimd.dma_start(xa[:], xT_rearr(xT_d, p))

                # ---- conv1: depthwise ----
                dwo1 = apool.tile([128, KC, 2 * SP], c1dt, tag="dwo1")
                for cc in range(KC):
                    # zero-padded interior [14, 14] per sample (walrus caps
                    # TensorScalarPtr / many APs at partition + 2 free dims)
                    src = pair_img(xa[:, cc, :])
                    for sl in range(2):
                        nc.scalar.copy(pads1[cc][:, sl, 1:15, 1:15], src[:, sl])
                    dwv = dwo1[:, cc, :].rearrange("p (s h w) -> p s h w", s=2, h=HW)
                    for t in range(9):
                        ky, kx = divmod(t, 3)
                        for sl in range(2):
                            win = pads1[cc][:, sl, ky:ky + 14, kx:kx + 14]
                            if t == 0:
                                nc.vector.tensor_scalar(
                                    dwv[:, sl], win, dw1w[:, cc, 0:1],
                                    dw1b[:, cc:cc + 1], Alu.mult, Alu.add)
                            else:
                                nc.vector.scalar_tensor_tensor(
                                    dwv[:, sl], win, dw1w[:, cc, t:t + 1],
                                    dwv[:, sl], Alu.mult, Alu.add)

                # ---- conv1: pointwise + residual (in-place on xa) ----
                for mc in range(KC):
                    ps1 = pmm.tile([128, 2 * SP], f32, tag="psmm")
                    for kc in range(KC):
                        nc.tensor.matmul(
                            ps1[:], pw1T[:, kc, mc * 128:(mc + 1) * 128],
                            dwo1[:, kc, :],
                            start=(kc == 0), stop=(kc == KC - 1))
                    xv = pair_inner(xa[:, mc, :])
                    nc.vector.scalar_tensor_tensor(
                        xv, ps1[:].rearrange("p (s n) -> p s n", s=2),
                        pw1b[:, mc:mc + 1], xv, Alu.add, Alu.add)
                    # cls token: x + cls = 2*x
                    cls = xa[:, mc, 0:PW:N]
                    nc.scalar.mul(cls, cls, 2.0)

                # ---- qkv + per-head attention pieces ----
                qT = apool.tile([128, KC, PW], f32, tag="qT")
                attnT = apool.tile([128, KC, PW], f32, tag="attnT")
                stats = apool.tile([128, KC, 8], f32, tag="stats")
                # stats[:, cc, :] = [S0, S1, ctx0, ctx1, Sr0, Sr1, cls_v0, cls_v1]
                dwo2 = apool.tile([128, KC, 2 * SP],
                                  bf16 if conv2_bf16 else mmdt, tag="dwo2")

                def qkv_mm(mc):
                    ps = pmm.tile([128, PW], f32, name=f"psq_{p}_{mc}", tag="psmm")
                    for kc in range(KC):
                        nc.tensor.matmul(
                            ps[:], mcast(wqkv[:, kc, mc * 128:(mc + 1) * 128]),
                            mcast(xa[:, kc, :]),
                            start=(kc == 0), stop=(kc == KC - 1))
                    return ps

                for mc in range(KC):  # q
                    ps = qkv_mm(mc)
                    nc.scalar.copy(qT[:, mc, :], ps[:])

                for cc in range(KC):  # k then v, per head-chunk
                    psk = qkv_mm(KC + cc)
                    ek = epool.tile([128, PW], f32, tag="expk")
                    for sl in range(2):
                        nc.scalar.activation(
                            ek[:, sl * N:(sl + 1) * N], psk[:, sl * N:(sl + 1) * N],
                            Act.Exp, accum_out=stats[:, cc, sl:sl + 1])
                    psv = qkv_mm(2 * KC + cc)
                    # conv2 padded interior from v
                    vimg = pair_img(psv)
                    for sl in range(2):
                        nc.scalar.copy(pads2[cc][:, sl, 1:15, 1:15], vimg[:, sl])
                    # cls_v
                    nc.scalar.copy(stats[:, cc, 6:8], psv[:, 0:PW:N])
                    # ctx_raw = sum_n expk * vT  (overwrites expk with the
                    # product; scalar_tensor_tensor's accum_out sums it)
                    for sl in range(2):
                        nc.vector.scalar_tensor_tensor(
                            ek[:, sl * N:(sl + 1) * N], ek[:, sl * N:(sl + 1) * N],
                            1.0, psv[:, sl * N:(sl + 1) * N],
                            Alu.mult, Alu.mult,
                            accum_out=stats[:, cc, 2 + sl:3 + sl])
                    # 1/S
                    nc.vector.reciprocal(stats[:, cc, 4:6], stats[:, cc, 0:2])
                    # attn = q * ctx_raw * (1/S)
                    for sl in range(2):
                        nc.vector.tensor_scalar(
                            attnT[:, cc, sl * N:(sl + 1) * N],
                            qT[:, cc, sl * N:(sl + 1) * N],
                            stats[:, cc, 2 + sl:3 + sl], stats[:, cc, 4 + sl:5 + sl],
                            Alu.mult, Alu.mult)

                nc.sync.dma_start(attn_rearr(attn_d, p), attnT[:])

                # ---- conv2: depthwise on v ----
                for cc in range(KC):
                    dwv = dwo2[:, cc, :].rearrange("p (s h w) -> p s h w", s=2, h=HW)
                    for t in range(9):
                        ky, kx = divmod(t, 3)
                        for sl in range(2):
                            win = pads2[cc][:, sl, ky:ky + 14, kx:kx + 14]
                            if t == 0:
                                nc.vector.tensor_scalar(
                                    dwv[:, sl], win, dw2w[:, cc, 0:1],
                                    dw2b[:, cc:cc + 1], Alu.mult, Alu.add)
                            else:
                                nc.vector.scalar_tensor_tensor(
                                    dwv[:, sl], win, dw2w[:, cc, t:t + 1],
                                    dwv[:, sl], Alu.mult, Alu.add)

                # ---- conv2: pointwise -> v_new; then out_pre = q*v_new + attn ----
                vnew = apool.tile([128, KC, PW], mmdt, tag="vnew")
                for mc in range(KC):
                    ps2 = pmm.tile([128, 2 * SP], f32, tag="psmm")
                    for kc in range(KC):
                        a = pw2T[:, kc, mc * 128:(mc + 1) * 128]
                        b = dwo2[:, kc, :]
                        if not conv2_bf16:
                            a, b = mcast(a), mcast(b)
                        nc.tensor.matmul(ps2[:], a, b,
                                         start=(kc == 0), stop=(kc == KC - 1))
                    nc.scalar.activation(
                        pair_inner(vnew[:, mc, :]),
                        ps2[:].rearrange("p (s n) -> p s n", s=2),
                        Act.Identity, bias=pw2b[:, mc:mc + 1])
                    nc.scalar.copy(vnew[:, mc, 0:PW:N], stats[:, mc, 6:8])
                    nc.vector.tensor_tensor(
                        vnew[:, mc, :], qT[:, mc, :], vnew[:, mc, :], op=Alu.mult)
                    nc.vector.tensor_tensor(
                        vnew[:, mc, :], vnew[:, mc, :], attnT[:, mc, :], op=Alu.add)

                # ---- final projection out = out_pre^T.T @ w_out + b_out ----
                for m_off, m_sz in ((0, 128), (128, 128), (256, 128), (384, 10)):
                    on = opool.tile([128, D], f32, tag="outn")
                    for n_off, n_sz in ((0, 512), (512, 256)):
                        po = pout.tile([128, 512], f32, tag="psout")
                        for kc in range(KC):
                            nc.tensor.matmul(
                                po[:m_sz, :n_sz],
                                mcast(vnew[:, kc, m_off:m_off + m_sz]),
                                mcast(wout[:, kc, n_off:n_off + n_sz]),
                                start=(kc == 0), stop=False)
                        nc.tensor.matmul(
                            po[:m_sz, :n_sz], mcast(ones[0:1, 0:m_sz]),
                            mcast(bout[0:1, n_off:n_off + n_sz]),
                            start=False, stop=True)
                        nc.scalar.copy(on[:m_sz, n_off:n_off + n_sz],
                                       po[:m_sz, :n_sz])
                    nc.sync.dma_start(
                        out_d.ap()[p * PW + m_off: p * PW + m_off + m_sz, :],
                        on[:m_sz, :])

    return nc


def xT_rearr(d, p=None):
    ap = d.ap() if p is None else d.ap()[p]
    return ap.rearrange("c p n -> p c n")


def attn_rearr(d, p):
    return d.ap()[p].rearrange("c p n -> p c n")


def pair_img(ap_2d):
    """[128, PW] -> [128, 2, 14, 14]: spatial tokens (skip cls) per sample."""
    return ap_2d.rearrange("p (s n) -> p s n", s=2)[:, :, 1:1 + SP].rearrange(
        "p s (h w) -> p s h w", h=HW)


def pair_inner(ap_2d):
    """[128, PW] -> [128, 2, 196]: spatial-token slices of both samples."""
    return ap_2d.rearrange("p (s n) -> p s n", s=2)[:, :, 1:1 + SP]


def _get_nc(**knobs):
    key = tuple(sorted(knobs.items()))
    if key not in _cache:
        _cache[key] = _build(**knobs)
    return _cache[key]


def _host_prep(inputs):
    """Build per-core input maps (host-side transposes are cheap numpy)."""
    x = np.ascontiguousarray(inputs["x"], dtype=np.float32)
    wqkv = np.ascontiguousarray(inputs["w_qkv"], dtype=np.float32)
    pw1 = inputs["pw1_w"].reshape(D, D).astype(np.float32)
    pw2 = inputs["pw2_w"].reshape(INNER, INNER).astype(np.float32)
    wout = np.ascontiguousarray(inputs["w_out"], dtype=np.float32)

    wqkv_c = np.ascontiguousarray(wqkv.reshape(KC, 128, 3 * INNER))
    pw1T = np.ascontiguousarray(pw1.T.reshape(KC, 128, D))
    pw2T = np.ascontiguousarray(pw2.T.reshape(KC, 128, D))
    wout_c = np.ascontiguousarray(wout.reshape(KC, 128, D))

    def chunk_pp(v, n=1):  # [768, n] -> [128, KC*n] partition-major
        return np.ascontiguousarray(
            v.reshape(KC, 128, -1).transpose(1, 0, 2).reshape(128, -1))

    dw1w = chunk_pp(inputs["dw1_w"].reshape(D, 9).astype(np.float32))
    dw2w = chunk_pp(inputs["dw2_w"].reshape(INNER, 9).astype(np.float32))
    biases = np.concatenate(
        [chunk_pp(inputs[k].reshape(D, 1).astype(np.float32))
         for k in ("dw1_b", "pw1_b", "dw2_b", "pw2_b")], axis=1)
    biases = np.ascontiguousarray(biases)
    bout = np.ascontiguousarray(inputs["b_out"].reshape(1, D).astype(np.float32))
    ones = np.ones((1, 128), dtype=np.float32)

    in_maps = []
    for c in range(NCORES):
        xs = x[c * BL:(c + 1) * BL]  # [8, 197, 768]
        xT = xs.transpose(0, 2, 1).reshape(NPAIR, 2, KC, 128, N)
        xT = np.ascontiguousarray(xT.transpose(0, 2, 3, 1, 4).reshape(
            NPAIR, KC, 128, PW))
        in_maps.append({
            "xT": xT, "w_qkv": wqkv_c, "pw1T": pw1T, "pw2T": pw2T,
            "w_out": wout_c, "dw1w": dw1w, "dw2w": dw2w, "biases": biases,
            "b_out": bout, "ones": ones,
        })
    return in_maps


def _host_post(results):
    outs, attns = [], []
    for r in results:
        out = r["out"].reshape(BL, N, D)
        outs.append(out)
        a = r["attn"].reshape(NPAIR, KC, 2, DH, 2, N)
        # [pair, cc, h_loc, dh, sl, n] -> [pair, sl, (cc h_loc), n, dh]
        a = a.transpose(0, 4, 1, 2, 5, 3).reshape(BL, HEADS, N, DH)
        attns.append(a)
    return (np.concatenate(outs, 0), np.concatenate(attns, 0))


def kernel(**inputs):
    from concourse.bass_utils import run_bass_kernel_spmd

    nc = _get_nc()
    in_maps = _host_prep(inputs)
    res = run_bass_kernel_spmd(nc, in_maps, core_ids=list(range(NCORES)))
    return _host_post(res.results)


def run_traced(inputs, reps=3, **knobs):
    """test-harness entry: returns ((out, attn), wall-times-per-run)."""
    import time

    from concourse.bass_utils import run_bass_kernel_spmd

    nc = _get_nc(**knobs)
    in_maps = _host_prep(inputs)
    res = run_bass_kernel_spmd(nc, in_maps, core_ids=list(range(NCORES)))
    walls = []
    for _ in range(reps):
        t0 = time.time()
        res = run_bass_kernel_spmd(nc, in_maps, core_ids=list(range(NCORES)))
        walls.append(time.time() - t0)
    return _host_post(res.results), walls
